# revision 21
# baseline (speedup 1.0000x reference)
"""Trainium2 Bass kernel for nn_C4Transformer (moe_routing).

Self-contained: builds a bass program at first call (constants derived
from the actual table inputs), shards N across 8 NeuronCores (pure data
parallel), runs via run_bass_kernel_spmd, gathers the full output.
"""
import numpy as np

N_TOTAL = 1 << 20
N_CORES = 8
N_CORE = N_TOTAL // N_CORES  # 131072
P = 128
F = N_CORE // P  # 1024

_cache = {}


def _derive_consts(log_keys, recip_values):
    k = np.asarray(log_keys, np.float64)
    v = np.asarray(recip_values, np.float64)
    n = len(k)
    hh = np.diff(k)
    h = float(np.mean(hh))
    c = 50.0 * h
    r = np.exp(-c)

    # exact prefix/suffix sums of the softmax mixture
    A = np.zeros(n); C = np.zeros(n); Bt = np.zeros(n); Dt = np.zeros(n)
    acc_a = 0.0; acc_c = 0.0
    for j in range(n):
        acc_a = acc_a * r + v[j]
        acc_c = acc_c * r + 1.0
        A[j] = acc_a; C[j] = acc_c
    acc_b = 0.0; acc_d = 0.0
    for j in range(n - 1, -1, -1):
        if j < n - 1:
            acc_b = (acc_b + v[j + 1]) * r
            acc_d = (acc_d + 1.0) * r
        Bt[j] = acc_b; Dt[j] = acc_d

    with np.errstate(divide="ignore"):
        lv = np.log(np.maximum(v, 1e-300))
    rho = float(np.exp(np.mean(np.diff(lv))))
    g = -np.log(rho)
    js = np.arange(n)
    mid = (js >= 5) & (js <= n - 6) if n > 12 else js >= 0
    cA = float(np.median(A[mid] / rho ** js[mid]))
    cB = float(np.median(Bt[mid] / rho ** js[mid]))
    cC = float(np.median(C[mid]))
    cD = float(np.median(Dt[mid]))

    # S(f) = (alpha + eB*gamma)/(alpha + eD*gamma) with alpha=e^{-cf},
    # gamma=e^{cf} is exactly a logistic:
    #   S = k1 + k2 * sigmoid(-(2c f + ln eD)), k1 = eB/eD, k2 = 1 - k1
    eB = cB / cA
    eD = cD / cC
    k1 = eB / eD
    k2 = 1.0 - k1
    ln_eD = np.log(eD)

    lnV = np.log(cA / cC)
    s_ln = 1.0 / (2.0 * np.log(2.0) * h)  # t = s_ln * ln(b^2) + c0
    c0 = -float(k[0]) / h
    t_hi = (n - 1) - 1.0 / 64.0
    return dict(h=h, c=c, g=g, lnV=lnV, s_ln=s_ln, c0=c0, t_hi=t_hi,
                k1=float(k1), k2=float(k2), ln_eD=float(ln_eD))


def _build(consts):
    import concourse.bass as bass
    import concourse.bacc as bacc
    import concourse.tile as tile
    import concourse.mybir as mybir

    OP = mybir.AluOpType
    AT = mybir.ActivationFunctionType
    f32 = mybir.dt.float32
    i32 = mybir.dt.int32
    i8 = mybir.dt.int8

    nc = bacc.Bacc(trn_type="TRN2")

    def reg_const(val):
        val = float(val)
        if (f32, val) in nc.const_aps.aps:
            return
        t = nc.alloc_sbuf_tensor(f"cst-{len(nc.const_aps.aps)}", [128, 1], f32)
        nc.gpsimd.memset(t.ap(), val)
        nc.const_aps.aps[(f32, val)] = t.ap()

    g = consts["g"]; c = consts["c"]; lnV = consts["lnV"]

    # activation biases used (silu biases, exp/sigmoid biases)
    for b_ in (20.0, -20.0, float(lnV), float(-consts["ln_eD"]), 0.0):
        reg_const(b_)

    a_d = nc.dram_tensor("a", [N_CORE], f32, kind="ExternalInput")
    b_d = nc.dram_tensor("b", [N_CORE], f32, kind="ExternalInput")
    o_d = nc.dram_tensor("opcode", [N_CORE], i32, kind="ExternalInput")
    y_d = nc.dram_tensor("out", [N_CORE], f32, kind="ExternalOutput")

    with tile.TileContext(nc) as tc:
        with tc.tile_pool(name="pl", bufs=1) as pool:
            av = a_d[:].rearrange("(p f) -> p f", p=P)
            bv = b_d[:].rearrange("(p f) -> p f", p=P)
            ov = o_d[:].rearrange("(p f) -> p f", p=P)
            yv = y_d[:].rearrange("(p f) -> p f", p=P)

            def T(nm, dt=f32):
                return pool.tile([P, F], dt, name=nm, tag=nm)

            a = T("a"); b = T("b"); opi = T("opi", i32)
            nc.sync.dma_start(opi[:], ov)
            nc.sync.dma_start(b[:], bv)
            nc.sync.dma_start(a[:], av)

            # ---- ACT front: div-chain transcendentals ----
            sq = T("sq"); lnb = T("lnb")
            nc.scalar.activation(sq[:], b[:], AT.Square, bias=0.0, scale=1.0)
            nc.scalar.activation(lnb[:], sq[:], AT.Ln, bias=0.0, scale=1.0)

            # ---- equality masks (DVE, int8) overlap the ACT front ----
            masks = {}
            for e in range(14, 30):
                m = pool.tile([P, F], i8, name=f"m{e}", tag=f"m{e}")
                nc.vector.tensor_scalar(m[:], opi[:], float(e), None, OP.is_equal)
                masks[e] = m

            out = T("out")
            nc.gpsimd.memset(out[:], 0.0)

            # ---- t = clamp(s*ln(b^2)+c0, 0, hi); j = rne(t); f = t - j ----
            t_ = T("t_")
            nc.vector.tensor_scalar(t_[:], lnb[:], consts["s_ln"], consts["c0"],
                                    OP.mult, OP.add)
            nc.vector.tensor_scalar(t_[:], t_[:], 0.0, consts["t_hi"], OP.max, OP.min)
            ji = T("ji", i32); jf = T("jf")
            nc.scalar.copy(ji[:], t_[:])
            nc.scalar.copy(jf[:], ji[:])

            # ---- d + int block on DVE while ACT handles ji/jf ----
            d = T("d")
            nc.vector.tensor_tensor(d[:], a[:], b[:], OP.subtract)
            ai = T("ai", i32); bi = T("bi", i32)
            nc.scalar.copy(ai[:], a[:])
            nc.scalar.copy(bi[:], b[:])

            # ACT: silus on d, then E/sig once jf/f ready
            u_p = T("u_p"); u_0 = T("u_0"); u_m = T("u_m")
            nc.scalar.activation(u_p[:], d[:], AT.Silu, bias=20.0, scale=20.0)
            nc.scalar.activation(u_0[:], d[:], AT.Silu, bias=0.0, scale=20.0)
            nc.scalar.activation(u_m[:], d[:], AT.Silu, bias=-20.0, scale=20.0)

            f_ = T("f_")
            nc.vector.tensor_tensor(f_[:], t_[:], jf[:], OP.subtract)
            E = T("E")
            nc.scalar.activation(E[:], jf[:], AT.Exp, bias=float(lnV), scale=-g)
            sig = T("Q")
            nc.scalar.activation(sig[:], f_[:], AT.Sigmoid,
                                 bias=float(-consts["ln_eD"]),
                                 scale=float(-2.0 * c))

            # DVE int ops
            xor_i = T("xor_i", i32); and_i = T("and_i", i32); or_i = T("bw", i32)
            nc.vector.tensor_tensor(xor_i[:], ai[:], bi[:], OP.bitwise_xor)
            nc.vector.tensor_tensor(and_i[:], ai[:], bi[:], OP.bitwise_and)
            nc.vector.tensor_tensor(or_i[:], ai[:], bi[:], OP.bitwise_or)
            shamt = T("shamt", i32)
            nc.vector.tensor_scalar(shamt[:], bi[:], 0, 31, OP.max, OP.min)
            shl_i = T("shl_i", i32); shr_i = T("shr_i", i32)
            nc.vector.tensor_tensor(shl_i[:], ai[:], shamt[:], OP.arith_shift_left)
            nc.vector.tensor_tensor(shr_i[:], ai[:], shamt[:], OP.arith_shift_right)
            or_f = T("out2"); xor_f = T("xor_f"); and_f = T("and_f")
            nc.scalar.copy(or_f[:], or_i[:])
            nc.scalar.copy(xor_f[:], xor_i[:])
            nc.scalar.copy(and_f[:], and_i[:])
            shl_f = T("shl_f"); shr_f = T("shr_f")
            nc.scalar.copy(shl_f[:], shl_i[:])
            nc.scalar.copy(shr_f[:], shr_i[:])

            # ---- comparison planes ----
            ge20 = T("ai")
            nc.vector.tensor_tensor(ge20[:], u_p[:], u_0[:], OP.subtract)
            gt20 = T("bi")
            nc.vector.tensor_tensor(gt20[:], u_0[:], u_m[:], OP.subtract)
            gev = T("gev"); gtv = T("gtv"); lev = T("lev"); ltv = T("ltv")
            nc.gpsimd.tensor_scalar(gev[:], ge20[:], 0.05, None, OP.mult)
            nc.gpsimd.tensor_scalar(gtv[:], gt20[:], 0.05, None, OP.mult)
            nc.gpsimd.tensor_scalar(lev[:], gtv[:], -1.0, 1.0, OP.mult, OP.add)
            nc.gpsimd.tensor_scalar(ltv[:], gev[:], -1.0, 1.0, OP.mult, OP.add)
            eqv = T("eqv"); nev = T("nev")
            nc.gpsimd.tensor_tensor(eqv[:], gev[:], lev[:], OP.mult)
            nc.gpsimd.tensor_scalar(nev[:], eqv[:], -1.0, 1.0, OP.mult, OP.add)
            addp = T("addp"); mulp = T("mulp")
            nc.gpsimd.tensor_tensor(addp[:], a[:], b[:], OP.add)
            nc.gpsimd.tensor_tensor(mulp[:], a[:], b[:], OP.mult)

            # ---- q chain ----
            Sv = T("Sv")
            nc.vector.tensor_scalar(Sv[:], sig[:], consts["k2"], consts["k1"],
                                    OP.mult, OP.add)
            q1 = T("lnb"); q2 = T("q2")
            nc.vector.tensor_tensor(q1[:], a[:], E[:], OP.mult)
            nc.vector.tensor_tensor(q2[:], q1[:], Sv[:], OP.mult)
            qs = T("qs")
            nc.vector.tensor_scalar(qs[:], q2[:], 0.5, None, OP.subtract)
            qi = T("qi", i32); qf = T("qf")
            nc.scalar.copy(qi[:], qs[:])
            nc.scalar.copy(qf[:], qi[:])

            # ---- early merge preds (fill DVE while ACT finishes qi/qf) ----
            for e, plane in ((23, shl_f), (24, shr_f), (26, d),
                             (14, or_f), (15, xor_f), (16, and_f),
                             (25, addp), (27, mulp),
                             (19, ltv), (20, gtv), (21, lev), (22, gev),
                             (17, eqv), (18, nev)):
                nc.vector.copy_predicated(out[:], masks[e][:], plane[:])

            # ---- div/mod tail ----
            bpos = T("u_m")
            nc.vector.tensor_scalar(bpos[:], b[:], 0.0, None, OP.is_gt)
            divp = T("sq")
            nc.vector.tensor_tensor(divp[:], qf[:], bpos[:], OP.mult)
            dm = T("u_p"); modp = T("modp"); bnz = T("u_0")
            nc.vector.tensor_tensor(dm[:], divp[:], b[:], OP.mult)
            nc.vector.tensor_scalar(bnz[:], b[:], 0.0, None, OP.not_equal)
            nc.vector.scalar_tensor_tensor(modp[:], dm[:], -1.0, a[:], OP.mult, OP.add)
            nc.vector.tensor_tensor(modp[:], modp[:], bnz[:], OP.mult)
            nc.vector.copy_predicated(out[:], masks[28][:], divp[:])
            nc.vector.copy_predicated(out[:], masks[29][:], modp[:])

            nc.sync.dma_start(yv, out[:])

    nc.compile()
    return nc


def _get_program(log_keys, recip_values):
    key = (log_keys.tobytes(), recip_values.tobytes())
    if key not in _cache:
        consts = _derive_consts(log_keys, recip_values)
        _cache[key] = (_build(consts), consts)
    return _cache[key]


def kernel(a, b, log_keys, recip_values, opcode):
    from concourse.bass_utils import run_bass_kernel_spmd

    a = np.ascontiguousarray(np.asarray(a, np.float32))
    b = np.ascontiguousarray(np.asarray(b, np.float32))
    opcode = np.ascontiguousarray(np.asarray(opcode, np.int32))
    log_keys = np.ascontiguousarray(np.asarray(log_keys, np.float32))
    recip_values = np.ascontiguousarray(np.asarray(recip_values, np.float32))

    nc, consts = _get_program(log_keys, recip_values)

    a_s = a.reshape(N_CORES, N_CORE)
    b_s = b.reshape(N_CORES, N_CORE)
    o_s = opcode.reshape(N_CORES, N_CORE)
    in_maps = [{"a": a_s[i], "b": b_s[i], "opcode": o_s[i]}
               for i in range(N_CORES)]
    res = run_bass_kernel_spmd(nc, in_maps, core_ids=list(range(N_CORES)))
    out = np.concatenate([res.results[i]["out"] for i in range(N_CORES)])
    return out.astype(np.float32)


# revision 22
# speedup vs baseline: 1.3632x; 1.3632x over previous
"""Trainium2 Bass kernel for nn_C4Transformer (moe_routing).

Self-contained: builds a bass program at first call (constants derived
from the actual table inputs), shards N across 8 NeuronCores (pure data
parallel), runs via run_bass_kernel_spmd, gathers the full output.
"""
import numpy as np

N_TOTAL = 1 << 20
N_CORES = 8
N_CORE = N_TOTAL // N_CORES  # 131072
P = 128
F = N_CORE // P  # 1024

_cache = {}


def _derive_consts(log_keys, recip_values):
    k = np.asarray(log_keys, np.float64)
    v = np.asarray(recip_values, np.float64)
    n = len(k)
    hh = np.diff(k)
    h = float(np.mean(hh))
    c = 50.0 * h
    r = np.exp(-c)

    # exact prefix/suffix sums of the softmax mixture
    A = np.zeros(n); C = np.zeros(n); Bt = np.zeros(n); Dt = np.zeros(n)
    acc_a = 0.0; acc_c = 0.0
    for j in range(n):
        acc_a = acc_a * r + v[j]
        acc_c = acc_c * r + 1.0
        A[j] = acc_a; C[j] = acc_c
    acc_b = 0.0; acc_d = 0.0
    for j in range(n - 1, -1, -1):
        if j < n - 1:
            acc_b = (acc_b + v[j + 1]) * r
            acc_d = (acc_d + 1.0) * r
        Bt[j] = acc_b; Dt[j] = acc_d

    with np.errstate(divide="ignore"):
        lv = np.log(np.maximum(v, 1e-300))
    rho = float(np.exp(np.mean(np.diff(lv))))
    g = -np.log(rho)
    js = np.arange(n)
    mid = (js >= 5) & (js <= n - 6) if n > 12 else js >= 0
    cA = float(np.median(A[mid] / rho ** js[mid]))
    cB = float(np.median(Bt[mid] / rho ** js[mid]))
    cC = float(np.median(C[mid]))
    cD = float(np.median(Dt[mid]))

    # S(f) = (alpha + eB*gamma)/(alpha + eD*gamma) with alpha=e^{-cf},
    # gamma=e^{cf} is exactly a logistic:
    #   S = k1 + k2 * sigmoid(-(2c f + ln eD)), k1 = eB/eD, k2 = 1 - k1
    eB = cB / cA
    eD = cD / cC
    k1 = eB / eD
    k2 = 1.0 - k1
    ln_eD = np.log(eD)

    lnV = np.log(cA / cC)
    s_ln = 1.0 / (2.0 * np.log(2.0) * h)  # t = s_ln * ln(b^2) + c0
    c0 = -float(k[0]) / h
    t_hi = (n - 1) - 1.0 / 64.0
    return dict(h=h, c=c, g=g, lnV=lnV, s_ln=s_ln, c0=c0, t_hi=t_hi,
                k1=float(k1), k2=float(k2), ln_eD=float(ln_eD))


def _build(consts):
    import concourse.bass as bass
    import concourse.bacc as bacc
    import concourse.tile as tile
    import concourse.mybir as mybir

    OP = mybir.AluOpType
    AT = mybir.ActivationFunctionType
    f32 = mybir.dt.float32
    i32 = mybir.dt.int32
    i8 = mybir.dt.int8

    nc = bacc.Bacc(trn_type="TRN2")

    def reg_const(val):
        val = float(val)
        if (f32, val) in nc.const_aps.aps:
            return
        t = nc.alloc_sbuf_tensor(f"cst-{len(nc.const_aps.aps)}", [128, 1], f32)
        nc.gpsimd.memset(t.ap(), val)
        nc.const_aps.aps[(f32, val)] = t.ap()

    g = consts["g"]; c = consts["c"]; lnV = consts["lnV"]

    # activation biases used (silu biases, exp/sigmoid biases)
    for b_ in (20.0, -20.0, float(lnV), float(-consts["ln_eD"]), 0.0):
        reg_const(b_)

    a_d = nc.dram_tensor("a", [N_CORE], f32, kind="ExternalInput")
    b_d = nc.dram_tensor("b", [N_CORE], f32, kind="ExternalInput")
    o_d = nc.dram_tensor("opcode", [N_CORE], i32, kind="ExternalInput")
    y_d = nc.dram_tensor("out", [N_CORE], f32, kind="ExternalOutput")

    with tile.TileContext(nc) as tc:
        with tc.tile_pool(name="pl", bufs=1) as pool:
            av = a_d[:].rearrange("(p f) -> p f", p=P)
            bv = b_d[:].rearrange("(p f) -> p f", p=P)
            ov = o_d[:].rearrange("(p f) -> p f", p=P)
            yv = y_d[:].rearrange("(p f) -> p f", p=P)

            def T(nm, dt=f32):
                return pool.tile([P, F], dt, name=nm, tag=nm)

            a = T("a"); b = T("b"); opi = T("opi", i32)
            nc.sync.dma_start(opi[:], ov)
            nc.sync.dma_start(b[:], bv)
            nc.sync.dma_start(a[:], av)

            # ---- ACT front: div-chain transcendentals ----
            sq = T("sq"); lnb = T("lnb")
            nc.scalar.activation(sq[:], b[:], AT.Square, bias=0.0, scale=1.0)
            nc.scalar.activation(lnb[:], sq[:], AT.Ln, bias=0.0, scale=1.0)

            # ---- equality masks (DVE, int8) overlap the ACT front ----
            masks = {}
            for e in range(14, 30):
                m = pool.tile([P, F], i8, name=f"m{e}", tag=f"m{e}")
                nc.vector.tensor_scalar(m[:], opi[:], float(e), None, OP.is_equal)
                masks[e] = m

            out = T("out")
            nc.gpsimd.memset(out[:], 0.0)

            # ---- t = clamp(s*ln(b^2)+c0, 0, hi); j = rne(t); f = t - j ----
            t_ = T("t_")
            nc.vector.tensor_scalar(t_[:], lnb[:], consts["s_ln"], consts["c0"],
                                    OP.mult, OP.add)
            nc.vector.tensor_scalar(t_[:], t_[:], 0.0, consts["t_hi"], OP.max, OP.min)
            ji = T("ji", i32); jf = T("jf")
            nc.scalar.copy(ji[:], t_[:])
            nc.scalar.copy(jf[:], ji[:])

            # ---- d + int block on DVE while ACT handles ji/jf ----
            d = T("d")
            nc.vector.tensor_tensor(d[:], a[:], b[:], OP.subtract)
            ai = T("ai", i32); bi = T("bi", i32)
            nc.scalar.copy(ai[:], a[:])
            nc.scalar.copy(bi[:], b[:])

            # ACT: silus on d, then E/sig once jf/f ready
            u_p = T("u_p"); u_0 = T("u_0"); u_m = T("u_m")
            nc.scalar.activation(u_p[:], d[:], AT.Silu, bias=20.0, scale=20.0)
            nc.scalar.activation(u_0[:], d[:], AT.Silu, bias=0.0, scale=20.0)
            nc.scalar.activation(u_m[:], d[:], AT.Silu, bias=-20.0, scale=20.0)

            f_ = T("f_")
            nc.vector.tensor_tensor(f_[:], t_[:], jf[:], OP.subtract)
            E = T("E")
            nc.scalar.activation(E[:], jf[:], AT.Exp, bias=float(lnV), scale=-g)
            sig = T("Q")
            nc.scalar.activation(sig[:], f_[:], AT.Sigmoid,
                                 bias=float(-consts["ln_eD"]),
                                 scale=float(-2.0 * c))

            # DVE int ops
            xor_i = T("xor_i", i32); and_i = T("and_i", i32); or_i = T("bw", i32)
            nc.vector.tensor_tensor(xor_i[:], ai[:], bi[:], OP.bitwise_xor)
            nc.vector.tensor_tensor(and_i[:], ai[:], bi[:], OP.bitwise_and)
            nc.vector.tensor_tensor(or_i[:], ai[:], bi[:], OP.bitwise_or)
            shamt = T("shamt", i32)
            nc.vector.tensor_scalar(shamt[:], bi[:], 0, 31, OP.max, OP.min)
            shl_i = T("shl_i", i32); shr_i = T("shr_i", i32)
            nc.vector.tensor_tensor(shl_i[:], ai[:], shamt[:], OP.arith_shift_left)
            nc.vector.tensor_tensor(shr_i[:], ai[:], shamt[:], OP.arith_shift_right)
            or_f = T("out2"); xor_f = T("xor_f"); and_f = T("and_f")
            nc.scalar.copy(or_f[:], or_i[:])
            nc.scalar.copy(xor_f[:], xor_i[:])
            nc.scalar.copy(and_f[:], and_i[:])
            shl_f = T("shl_f"); shr_f = T("shr_f")
            nc.scalar.copy(shl_f[:], shl_i[:])
            nc.scalar.copy(shr_f[:], shr_i[:])

            # ---- comparison planes ----
            ge20 = T("ai")
            nc.vector.tensor_tensor(ge20[:], u_p[:], u_0[:], OP.subtract)
            gt20 = T("bi")
            nc.vector.tensor_tensor(gt20[:], u_0[:], u_m[:], OP.subtract)
            gev = T("gev"); gtv = T("gtv"); lev = T("lev"); ltv = T("ltv")
            nc.vector.tensor_scalar(gev[:], ge20[:], 0.05, None, OP.mult)
            nc.vector.tensor_scalar(gtv[:], gt20[:], 0.05, None, OP.mult)
            nc.vector.tensor_scalar(lev[:], gtv[:], -1.0, 1.0, OP.mult, OP.add)
            nc.vector.tensor_scalar(ltv[:], gev[:], -1.0, 1.0, OP.mult, OP.add)
            eqv = T("eqv"); nev = T("nev")
            nc.gpsimd.tensor_tensor(eqv[:], gev[:], lev[:], OP.mult)
            nc.gpsimd.tensor_scalar(nev[:], eqv[:], -1.0, 1.0, OP.mult, OP.add)
            addp = T("addp"); mulp = T("mulp")
            nc.gpsimd.tensor_tensor(addp[:], a[:], b[:], OP.add)
            nc.gpsimd.tensor_tensor(mulp[:], a[:], b[:], OP.mult)

            # ---- q chain ----
            Sv = T("Sv")
            nc.vector.tensor_scalar(Sv[:], sig[:], consts["k2"], consts["k1"],
                                    OP.mult, OP.add)
            q1 = T("lnb"); q2 = T("q2")
            nc.vector.tensor_tensor(q1[:], a[:], E[:], OP.mult)
            nc.vector.tensor_tensor(q2[:], q1[:], Sv[:], OP.mult)
            qs = T("qs")
            nc.vector.tensor_scalar(qs[:], q2[:], 0.5, None, OP.subtract)
            qi = T("qi", i32); qf = T("qf")
            nc.scalar.copy(qi[:], qs[:])
            nc.scalar.copy(qf[:], qi[:])

            # ---- early merge preds (fill DVE while ACT finishes qi/qf) ----
            for e, plane in ((23, shl_f), (24, shr_f), (26, d),
                             (14, or_f), (15, xor_f), (16, and_f),
                             (25, addp), (27, mulp),
                             (19, ltv), (20, gtv), (21, lev), (22, gev),
                             (17, eqv), (18, nev)):
                nc.vector.copy_predicated(out[:], masks[e][:], plane[:])

            # ---- div/mod tail ----
            bpos = T("u_m")
            nc.vector.tensor_scalar(bpos[:], b[:], 0.0, None, OP.is_gt)
            divp = T("sq")
            nc.vector.tensor_tensor(divp[:], qf[:], bpos[:], OP.mult)
            dm = T("u_p"); modp = T("modp"); bnz = T("u_0")
            nc.vector.tensor_tensor(dm[:], divp[:], b[:], OP.mult)
            nc.vector.tensor_scalar(bnz[:], b[:], 0.0, None, OP.not_equal)
            nc.vector.scalar_tensor_tensor(modp[:], dm[:], -1.0, a[:], OP.mult, OP.add)
            nc.vector.tensor_tensor(modp[:], modp[:], bnz[:], OP.mult)
            nc.vector.copy_predicated(out[:], masks[28][:], divp[:])
            nc.vector.copy_predicated(out[:], masks[29][:], modp[:])

            nc.sync.dma_start(yv, out[:])

    nc.compile()
    return nc


def _get_program(log_keys, recip_values):
    key = (log_keys.tobytes(), recip_values.tobytes())
    if key not in _cache:
        consts = _derive_consts(log_keys, recip_values)
        _cache[key] = (_build(consts), consts)
    return _cache[key]


def kernel(a, b, log_keys, recip_values, opcode):
    from concourse.bass_utils import run_bass_kernel_spmd

    a = np.ascontiguousarray(np.asarray(a, np.float32))
    b = np.ascontiguousarray(np.asarray(b, np.float32))
    opcode = np.ascontiguousarray(np.asarray(opcode, np.int32))
    log_keys = np.ascontiguousarray(np.asarray(log_keys, np.float32))
    recip_values = np.ascontiguousarray(np.asarray(recip_values, np.float32))

    nc, consts = _get_program(log_keys, recip_values)

    a_s = a.reshape(N_CORES, N_CORE)
    b_s = b.reshape(N_CORES, N_CORE)
    o_s = opcode.reshape(N_CORES, N_CORE)
    in_maps = [{"a": a_s[i], "b": b_s[i], "opcode": o_s[i]}
               for i in range(N_CORES)]
    res = run_bass_kernel_spmd(nc, in_maps, core_ids=list(range(N_CORES)))
    out = np.concatenate([res.results[i]["out"] for i in range(N_CORES)])
    return out.astype(np.float32)


# revision 24
# speedup vs baseline: 1.3642x; 1.0007x over previous
"""Trainium2 Bass kernel for nn_C4Transformer (moe_routing).

Self-contained: builds a bass program at first call (constants derived
from the actual table inputs), shards N across 8 NeuronCores (pure data
parallel), runs via run_bass_kernel_spmd, gathers the full output.
"""
import numpy as np

N_TOTAL = 1 << 20
N_CORES = 8
N_CORE = N_TOTAL // N_CORES  # 131072
P = 128
F = N_CORE // P  # 1024

_cache = {}


def _derive_consts(log_keys, recip_values):
    k = np.asarray(log_keys, np.float64)
    v = np.asarray(recip_values, np.float64)
    n = len(k)
    hh = np.diff(k)
    h = float(np.mean(hh))
    c = 50.0 * h
    r = np.exp(-c)

    # exact prefix/suffix sums of the softmax mixture
    A = np.zeros(n); C = np.zeros(n); Bt = np.zeros(n); Dt = np.zeros(n)
    acc_a = 0.0; acc_c = 0.0
    for j in range(n):
        acc_a = acc_a * r + v[j]
        acc_c = acc_c * r + 1.0
        A[j] = acc_a; C[j] = acc_c
    acc_b = 0.0; acc_d = 0.0
    for j in range(n - 1, -1, -1):
        if j < n - 1:
            acc_b = (acc_b + v[j + 1]) * r
            acc_d = (acc_d + 1.0) * r
        Bt[j] = acc_b; Dt[j] = acc_d

    with np.errstate(divide="ignore"):
        lv = np.log(np.maximum(v, 1e-300))
    rho = float(np.exp(np.mean(np.diff(lv))))
    g = -np.log(rho)
    js = np.arange(n)
    mid = (js >= 5) & (js <= n - 6) if n > 12 else js >= 0
    cA = float(np.median(A[mid] / rho ** js[mid]))
    cB = float(np.median(Bt[mid] / rho ** js[mid]))
    cC = float(np.median(C[mid]))
    cD = float(np.median(Dt[mid]))

    # S(f) = (alpha + eB*gamma)/(alpha + eD*gamma) with alpha=e^{-cf},
    # gamma=e^{cf} is exactly a logistic:
    #   S = k1 + k2 * sigmoid(-(2c f + ln eD)), k1 = eB/eD, k2 = 1 - k1
    eB = cB / cA
    eD = cD / cC
    k1 = eB / eD
    k2 = 1.0 - k1
    ln_eD = np.log(eD)

    lnV = np.log(cA / cC)
    s_ln = 1.0 / (2.0 * np.log(2.0) * h)  # t = s_ln * ln(b^2) + c0
    c0 = -float(k[0]) / h
    t_hi = (n - 1) - 1.0 / 64.0
    return dict(h=h, c=c, g=g, lnV=lnV, s_ln=s_ln, c0=c0, t_hi=t_hi,
                k1=float(k1), k2=float(k2), ln_eD=float(ln_eD))


def _build(consts):
    import concourse.bass as bass
    import concourse.bacc as bacc
    import concourse.tile as tile
    import concourse.mybir as mybir

    OP = mybir.AluOpType
    AT = mybir.ActivationFunctionType
    f32 = mybir.dt.float32
    i32 = mybir.dt.int32
    i8 = mybir.dt.int8

    nc = bacc.Bacc(trn_type="TRN2")

    def reg_const(val):
        val = float(val)
        if (f32, val) in nc.const_aps.aps:
            return
        t = nc.alloc_sbuf_tensor(f"cst-{len(nc.const_aps.aps)}", [128, 1], f32)
        nc.gpsimd.memset(t.ap(), val)
        nc.const_aps.aps[(f32, val)] = t.ap()

    g = consts["g"]; c = consts["c"]; lnV = consts["lnV"]

    # activation biases used (silu biases, exp/sigmoid biases)
    for b_ in (20.0, -20.0, float(lnV), float(-consts["ln_eD"]), 0.0):
        reg_const(b_)

    a_d = nc.dram_tensor("a", [N_CORE], f32, kind="ExternalInput")
    b_d = nc.dram_tensor("b", [N_CORE], f32, kind="ExternalInput")
    o_d = nc.dram_tensor("opcode", [N_CORE], i32, kind="ExternalInput")
    y_d = nc.dram_tensor("out", [N_CORE], f32, kind="ExternalOutput")

    with tile.TileContext(nc) as tc:
        with tc.tile_pool(name="pl", bufs=1) as pool:
            av = a_d[:].rearrange("(p f) -> p f", p=P)
            bv = b_d[:].rearrange("(p f) -> p f", p=P)
            ov = o_d[:].rearrange("(p f) -> p f", p=P)
            yv = y_d[:].rearrange("(p f) -> p f", p=P)

            def T(nm, dt=f32):
                return pool.tile([P, F], dt, name=nm, tag=nm)

            a = T("a"); b = T("b"); opi = T("opi", i32)
            nc.sync.dma_start(opi[:], ov)
            nc.sync.dma_start(b[:], bv)
            nc.sync.dma_start(a[:], av)

            # ---- ACT front: div-chain transcendentals ----
            sq = T("sq"); lnb = T("lnb")
            nc.scalar.activation(sq[:], b[:], AT.Square, bias=0.0, scale=1.0)
            nc.scalar.activation(lnb[:], sq[:], AT.Ln, bias=0.0, scale=1.0)

            # ---- equality masks (DVE, int8) overlap the ACT front ----
            masks = {}
            for e in range(14, 30):
                m = pool.tile([P, F], i8, name=f"m{e}", tag=f"m{e}")
                nc.vector.tensor_scalar(m[:], opi[:], float(e), None, OP.is_equal)
                masks[e] = m

            out = T("out")
            nc.gpsimd.memset(out[:], 0.0)

            # ---- t = clamp(s*ln(b^2)+c0, 0, hi); j = rne(t); f = t - j ----
            t_ = T("t_")
            nc.vector.tensor_scalar(t_[:], lnb[:], consts["s_ln"], consts["c0"],
                                    OP.mult, OP.add)
            nc.vector.tensor_scalar(t_[:], t_[:], 0.0, consts["t_hi"], OP.max, OP.min)
            ji = T("ji", i32); jf = T("jf")
            nc.scalar.copy(ji[:], t_[:])
            nc.scalar.copy(jf[:], ji[:])

            # ---- d + int block on DVE while ACT handles ji/jf ----
            d = T("d")
            nc.vector.tensor_tensor(d[:], a[:], b[:], OP.subtract)
            ai = T("ai", i32); bi = T("bi", i32)
            nc.scalar.copy(ai[:], a[:])
            nc.scalar.copy(bi[:], b[:])

            # ACT: silus on d, then E/sig once jf/f ready
            u_p = T("u_p"); u_0 = T("u_0"); u_m = T("u_m")
            nc.scalar.activation(u_p[:], d[:], AT.Silu, bias=20.0, scale=20.0)
            nc.scalar.activation(u_0[:], d[:], AT.Silu, bias=0.0, scale=20.0)
            nc.scalar.activation(u_m[:], d[:], AT.Silu, bias=-20.0, scale=20.0)

            f_ = T("f_")
            nc.vector.tensor_tensor(f_[:], t_[:], jf[:], OP.subtract)
            E = T("E")
            nc.scalar.activation(E[:], jf[:], AT.Exp, bias=float(lnV), scale=-g)
            sig = T("Q")
            nc.scalar.activation(sig[:], f_[:], AT.Sigmoid,
                                 bias=float(-consts["ln_eD"]),
                                 scale=float(-2.0 * c))

            # DVE int ops
            xor_i = T("xor_i", i32); and_i = T("and_i", i32); or_i = T("bw", i32)
            nc.vector.tensor_tensor(xor_i[:], ai[:], bi[:], OP.bitwise_xor)
            nc.vector.tensor_tensor(and_i[:], ai[:], bi[:], OP.bitwise_and)
            nc.vector.tensor_tensor(or_i[:], ai[:], bi[:], OP.bitwise_or)
            shamt = T("shamt", i32)
            nc.vector.tensor_scalar(shamt[:], bi[:], 0, 31, OP.max, OP.min)
            shl_i = T("shl_i", i32); shr_i = T("shr_i", i32)
            nc.vector.tensor_tensor(shl_i[:], ai[:], shamt[:], OP.arith_shift_left)
            nc.vector.tensor_tensor(shr_i[:], ai[:], shamt[:], OP.arith_shift_right)
            or_f = T("out2"); xor_f = T("xor_f"); and_f = T("and_f")
            nc.scalar.copy(or_f[:], or_i[:])
            nc.scalar.copy(xor_f[:], xor_i[:])
            nc.scalar.copy(and_f[:], and_i[:])
            shl_f = T("shl_f"); shr_f = T("shr_f")
            nc.scalar.copy(shl_f[:], shl_i[:])
            nc.scalar.copy(shr_f[:], shr_i[:])

            # ---- comparison planes ----
            ge20 = T("ai")
            nc.vector.tensor_tensor(ge20[:], u_p[:], u_0[:], OP.subtract)
            gt20 = T("bi")
            nc.vector.tensor_tensor(gt20[:], u_0[:], u_m[:], OP.subtract)
            gev = T("gev"); gtv = T("gtv"); lev = T("lev"); ltv = T("ltv")
            nc.vector.tensor_scalar(gev[:], ge20[:], 0.05, None, OP.mult)
            nc.vector.tensor_scalar(gtv[:], gt20[:], 0.05, None, OP.mult)
            nc.vector.tensor_scalar(lev[:], gtv[:], -1.0, 1.0, OP.mult, OP.add)
            nc.vector.tensor_scalar(ltv[:], gev[:], -1.0, 1.0, OP.mult, OP.add)
            addp = T("addp"); mulp = T("mulp")
            nc.gpsimd.tensor_tensor(addp[:], a[:], b[:], OP.add)
            nc.gpsimd.tensor_tensor(mulp[:], a[:], b[:], OP.mult)
            eqv = T("eqv"); nev = T("nev")
            nc.gpsimd.tensor_tensor(eqv[:], gev[:], lev[:], OP.mult)
            nc.gpsimd.tensor_scalar(nev[:], eqv[:], -1.0, 1.0, OP.mult, OP.add)

            # ---- q chain ----
            Sv = T("Sv")
            nc.vector.tensor_scalar(Sv[:], sig[:], consts["k2"], consts["k1"],
                                    OP.mult, OP.add)
            q1 = T("lnb"); q2 = T("q2")
            nc.vector.tensor_tensor(q1[:], a[:], E[:], OP.mult)
            nc.vector.tensor_tensor(q2[:], q1[:], Sv[:], OP.mult)
            qs = T("qs")
            nc.vector.tensor_scalar(qs[:], q2[:], 0.5, None, OP.subtract)
            qi = T("qi", i32); qf = T("qf")
            nc.scalar.copy(qi[:], qs[:])
            nc.scalar.copy(qf[:], qi[:])

            # ---- early merge preds (fill DVE while ACT finishes qi/qf) ----
            for e, plane in ((23, shl_f), (24, shr_f), (26, d),
                             (14, or_f), (15, xor_f), (16, and_f),
                             (25, addp), (27, mulp),
                             (19, ltv), (20, gtv), (21, lev), (22, gev),
                             (17, eqv), (18, nev)):
                nc.vector.copy_predicated(out[:], masks[e][:], plane[:])

            # ---- div/mod tail ----
            bpos = T("u_m")
            nc.vector.tensor_scalar(bpos[:], b[:], 0.0, None, OP.is_gt)
            divp = T("sq")
            nc.vector.tensor_tensor(divp[:], qf[:], bpos[:], OP.mult)
            dm = T("u_p"); modp = T("modp"); bnz = T("u_0")
            nc.vector.tensor_tensor(dm[:], divp[:], b[:], OP.mult)
            nc.vector.tensor_scalar(bnz[:], b[:], 0.0, None, OP.not_equal)
            nc.vector.scalar_tensor_tensor(modp[:], dm[:], -1.0, a[:], OP.mult, OP.add)
            nc.vector.tensor_tensor(modp[:], modp[:], bnz[:], OP.mult)
            nc.vector.copy_predicated(out[:], masks[28][:], divp[:])
            nc.vector.copy_predicated(out[:], masks[29][:], modp[:])

            nc.sync.dma_start(yv, out[:])

    nc.compile()
    return nc


def _get_program(log_keys, recip_values):
    key = (log_keys.tobytes(), recip_values.tobytes())
    if key not in _cache:
        consts = _derive_consts(log_keys, recip_values)
        _cache[key] = (_build(consts), consts)
    return _cache[key]


def kernel(a, b, log_keys, recip_values, opcode):
    from concourse.bass_utils import run_bass_kernel_spmd

    a = np.ascontiguousarray(np.asarray(a, np.float32))
    b = np.ascontiguousarray(np.asarray(b, np.float32))
    opcode = np.ascontiguousarray(np.asarray(opcode, np.int32))
    log_keys = np.ascontiguousarray(np.asarray(log_keys, np.float32))
    recip_values = np.ascontiguousarray(np.asarray(recip_values, np.float32))

    nc, consts = _get_program(log_keys, recip_values)

    a_s = a.reshape(N_CORES, N_CORE)
    b_s = b.reshape(N_CORES, N_CORE)
    o_s = opcode.reshape(N_CORES, N_CORE)
    in_maps = [{"a": a_s[i], "b": b_s[i], "opcode": o_s[i]}
               for i in range(N_CORES)]
    res = run_bass_kernel_spmd(nc, in_maps, core_ids=list(range(N_CORES)))
    out = np.concatenate([res.results[i]["out"] for i in range(N_CORES)])
    return out.astype(np.float32)
